# revision 39
# baseline (speedup 1.0000x reference)
"""KNN classifier (x[2048,512] vs keys[65536,512], top-10 mode vote).

Device (8 NeuronCores, datastore sharded 8192 keys/core):
  u = x @ keys_c.T in fp16 (PE, fp32 PSUM accum), dumped to HBM as fp16,
  plus per-128-key-chunk max of u (DVE) for cheap host-side candidate
  chunk selection.

Host: keys pre-sorted by |k|^2 so each 128-key chunk has a tight bias
range; chunks are selected by the provable bound max_u - 0.5*min|k|^2,
candidates narrowed with dumped fp16 scores, and the final top-10 is
re-ranked with exact f32 dot products so fp16 noise cannot flip labels.
"""

import os
import sys

import numpy as np

for _p in ("/opt/trn_rl_repo", "/root/.axon_site/_ro/trn_rl_repo"):
    if _p not in sys.path and os.path.isdir(_p):
        sys.path.append(_p)

N, D, M = 2048, 512, 65536
NCORES = 8
ML = M // NCORES          # 8192 keys per core
KT = D // 128             # 4 contraction tiles
NQ = N // 128             # 16 query tiles
CH = 512                  # psum chunk (one bank)
NCHUNK = ML // CH         # 16
CMW = 128                 # chunk-max granularity (keys per chunk)
NCM = ML // CMW           # 64 chunk maxes per core
TOPK = 10
NUM_CLASSES = 100
TOP_CHUNKS = 48           # candidate chunks per query (>= 12 guaranteed)
REFINE = 96               # exact-rescored candidates per query
QB = 4                    # query tiles per psum group (4 banks)

LAST_EXEC_NS = None


# --- walrus wait-cap workaround -------------------------------------------
# This toolchain's walrus caps semaphore waits per instruction at 1 (2 for
# EventSemaphore) but the Tile scheduler attaches more. Rewrite the BIR
# JSON before compile: excess waits move onto wait-only EventSemaphore
# instructions inserted before the oversubscribed instruction (same
# engine, in-order execution keeps semantics).
_WSPLIT = [0]


def _split_waits_json(bir):
    for fn in bir.get("functions", []):
        for bb in fn.get("blocks", []):
            out = []
            for inst in bb.get("instructions", []):
                si = inst.get("sync_info")
                if si:
                    waits = si.get("on_wait") or []
                    cap = 2 if inst.get("opcode") == "EventSemaphore" else 1
                    movable = [w for w in waits
                               if w.get("sync_type") == "semaphore"]
                    fixed = [w for w in waits
                             if w.get("sync_type") != "semaphore"]
                    room = max(cap - len(fixed), 0)
                    if len(movable) > room:
                        keep = movable[-room:] if room else []
                        excess = movable[:-room] if room else movable
                        si["on_wait"] = fixed + keep
                        for i in range(0, len(excess), 2):
                            _WSPLIT[0] += 1
                            out.append({
                                "engine": inst["engine"],
                                "ins": [], "outs": [],
                                "name": f"WSPLIT-{_WSPLIT[0]}",
                                "opcode": "EventSemaphore",
                                "debug": inst.get("debug", 0),
                                "sync_info": {"on_update": [],
                                              "on_wait": excess[i:i + 2]},
                            })
                out.append(inst)
            bb["instructions"] = out
    return bir


def _apply_tile_exit_patch():
    """Single-shot NEFF: skip the end-of-kernel semaphore clear + second
    barrier (the NEFF is loaded fresh per call, so sems never need
    resetting for a re-run). Keeps the allocator bookkeeping."""
    from concourse.tile import ScopedClock, TileContext

    if getattr(TileContext, "_exit_trimmed", False):
        return

    def _drain_and_barrier(self, tick_clock, wait_clock):
        drain_inst = self.nc.sync.drain()
        wait_clock.add_sem_waits(
            drain_inst.ins, ScopedClock({None: tick_clock.global_clock}))
        self.nc.all_engine_barrier()
        popped = self.nc._tile_sem_poison_stack.pop()
        assert popped is self._sem_poison
        sems = list(self.sems.allocated().values())
        if sems:
            nums = [s.num if hasattr(s, "num") else s for s in sems]
            self.nc._state.prepend_free_semaphores(nums)
            for poison_set in self.nc._tile_sem_poison_stack:
                poison_set.update(nums)

    TileContext._drain_and_barrier = _drain_and_barrier
    TileContext._exit_trimmed = True


def _apply_bir_patch():
    import json

    import concourse.bass2jax as b2j
    import concourse.bass_utils as bu

    if getattr(bu, "_wait_split_patched", False):
        return
    orig = bu.compile_bir_kernel

    def wrapper(bir_json, tmpdir, neff_name="file.neff", **kw):
        bir = json.loads(bytes(bir_json) if isinstance(
            bir_json, (bytes, bytearray)) else str(bir_json))
        bir = _split_waits_json(bir)
        return orig(json.dumps(bir).encode(), tmpdir, neff_name, **kw)

    bu.compile_bir_kernel = wrapper
    bu._wait_split_patched = True
    if getattr(b2j, "compile_bir_kernel", None) is orig:
        b2j.compile_bir_kernel = wrapper


# --- device kernel ---------------------------------------------------------
def _build_bass():
    import concourse.bass as bass
    import concourse.mybir as mybir
    from concourse.tile import TileContext

    nc = bass.Bass(trn_type="TRN2")
    f8 = mybir.dt.float8e4
    f16 = mybir.dt.float16
    f32 = mybir.dt.float32
    xT_d = nc.dram_tensor("xT", [D, N], f8, kind="ExternalInput")
    kT_d = nc.dram_tensor("kT", [D, ML], f8, kind="ExternalInput")
    # pair-maxes (keys b and b+64 of each 128-key chunk), halves the dump
    s_d = nc.dram_tensor("s", [N, ML // 2], f16, kind="ExternalOutput")
    # partition-major, chunk-major so each outer chunk flushes its own
    # 128x128B slice; host transposes back to [N, NCM]
    cm_d = nc.dram_tensor("cm", [128, NCM * NQ], f16, kind="ExternalOutput")

    CPC = CH // CMW  # chunk-maxes per psum chunk (4)

    NG = NQ // QB  # psum groups per chunk

    with TileContext(nc) as tc:
        with tc.tile_pool(name="inp", bufs=1) as inp, \
             tc.tile_pool(name="outp", bufs=2) as outp, \
             tc.tile_pool(name="ldr", bufs=2) as ldr, \
             tc.tile_pool(name="l1p", bufs=3) as l1p, \
             tc.tile_pool(name="ps", bufs=4, space="PSUM") as ps:
            # inputs arrive in fine slices, ordered so the first matmul's
            # operands (xt tile 0, key chunk 0) land first
            xts, kts = [], []

            def load_x(q):
                xq = inp.tile([128, KT, 128], f8, tag=f"xt{q}")
                nc.sync.dma_start(
                    out=xq[:],
                    in_=xT_d.ap()[:, q * 128:(q + 1) * 128]
                    .rearrange("(t p) n -> p t n", p=128))
                xts.append(xq)

            def load_k(c):
                kc = inp.tile([128, KT, CH], f8, tag=f"kt{c}")
                nc.sync.dma_start(
                    out=kc[:],
                    in_=kT_d.ap()[:, c * CH:(c + 1) * CH]
                    .rearrange("(t p) m -> p t m", p=128))
                kts.append(kc)

            load_x(0)
            load_k(0)
            load_k(1)
            for q in range(1, NQ):
                load_x(q)
            for c in range(2, NCHUNK):
                load_k(c)

            def ladder(dst, src, width, nq):
                # chunk-max via fp16 2x tensor_tensor max ladder:
                # width -> w/2 -> w/4 -> w/8, then a final reduce into dst.
                # Returns the first-stage (pair-max) tile, which doubles as
                # the score dump.
                cur = src
                w = width
                first = None
                while w > 16:
                    pool = l1p if w == width else ldr
                    nxt = pool.tile([128, nq, CPC, w // 2], f16,
                                    tag=f"l{w}_{nq}", name=f"lt{w}_{nq}")
                    nc.vector.tensor_tensor(
                        nxt[:], cur[:, :, :, 0:w // 2],
                        cur[:, :, :, w // 2:w], op=mybir.AluOpType.max)
                    if first is None:
                        first = nxt
                    cur, w = nxt, w // 2
                nc.vector.tensor_reduce(
                    dst, cur[:], axis=mybir.AxisListType.X,
                    op=mybir.AluOpType.max)
                return first

            HB = 2          # query tiles per psum half-tile (2 banks)
            NH = NQ // HB   # 8 half-tiles per chunk

            for c in range(NCHUNK):
                last = c == NCHUNK - 1
                # per-chunk score buffer + chunk-max tile (no cross-chunk
                # WAR on a shared accumulator)
                ev = outp.tile([128, NQ, CH], f16, tag="ev")
                cmc = ldr.tile([128, CPC, NQ], f16, tag="cmc")
                for h in range(NH):
                    acc = ps.tile([128, HB, CH], f32, tag="acc")
                    for j in range(HB):
                        q = h * HB + j
                        for t in range(KT // 2):
                            nc.tensor.matmul(
                                acc[:, j, :],
                                xts[q][:, 2 * t:2 * t + 2, :],
                                kts[c][:, 2 * t:2 * t + 2, :],
                                start=(t == 0), stop=(t == KT // 2 - 1),
                                perf_mode=mybir.MatmulPerfMode.DoubleRow)
                    # PSUM->SBUF fp16; DVE takes 1-2 of 8 (parity-based),
                    # Act the rest. Last chunk: all on Act so the DVE
                    # ladders drain the tail without queueing.
                    if not last and (h == 4 or (h == 5 and c % 2 == 0)):
                        nc.vector.tensor_copy(
                            ev[:, h * HB:(h + 1) * HB, :], acc[:])
                    else:
                        nc.scalar.copy(ev[:, h * HB:(h + 1) * HB, :],
                                       acc[:])
                    if last and h % 2 == 1:
                        g = h // 2
                        # tail: pair-max only, per 4 qtiles; the host
                        # derives this chunk's maxes from the dump
                        pm = l1p.tile([128, QB, CPC, CMW // 2], f16,
                                      tag="pml")
                        evg = ev[:, g * QB:(g + 1) * QB, :].rearrange(
                            "p q (a b) -> p q a b", b=CMW)
                        nc.vector.tensor_tensor(
                            pm[:], evg[:, :, :, 0:CMW // 2],
                            evg[:, :, :, CMW // 2:CMW],
                            op=mybir.AluOpType.max)
                        nc.sync.dma_start(
                            out=s_d.ap()[g * QB * 128:(g + 1) * QB * 128,
                                         c * (CH // 2):(c + 1) * (CH // 2)]
                            .rearrange("(q p) m -> p q m", p=128),
                            in_=pm[:].rearrange("p q a b -> p q (a b)"))
                if not last:
                    if c == NCHUNK - 2:
                        # shorten the tail: ladder per 4 qtiles so pieces
                        # start as their copies land
                        for g in range(NG):
                            pm = ladder(cmc[:, :, g * QB:(g + 1) * QB]
                                        .rearrange("p a q -> p q a"),
                                        ev[:, g * QB:(g + 1) * QB, :]
                                        .rearrange("p q (a b) -> p q a b",
                                                   b=CMW),
                                        CMW, QB)
                            nc.sync.dma_start(
                                out=s_d.ap()[
                                    g * QB * 128:(g + 1) * QB * 128,
                                    c * (CH // 2):(c + 1) * (CH // 2)]
                                .rearrange("(q p) m -> p q m", p=128),
                                in_=pm[:].rearrange("p q a b -> p q (a b)"))
                    else:
                        pm = ladder(cmc[:].rearrange("p a q -> p q a"),
                                    ev[:].rearrange("p q (a b) -> p q a b",
                                                    b=CMW),
                                    CMW, NQ)
                        nc.sync.dma_start(
                            out=s_d.ap()[:,
                                         c * (CH // 2):(c + 1) * (CH // 2)]
                            .rearrange("(q p) m -> p q m", p=128),
                            in_=pm[:].rearrange("p q a b -> p q (a b)"))
                    nc.sync.dma_start(
                        out=cm_d.ap()[:, c * CPC * NQ:(c + 1) * CPC * NQ],
                        in_=cmc[:].rearrange("p a q -> p (a q)"))
    return nc


def _device_scores(xT16, kT16_list):
    global LAST_EXEC_NS
    from concourse import bass_utils

    _apply_bir_patch()
    _apply_tile_exit_patch()
    nc = _build_bass()
    in_maps = [{"xT": xT16, "kT": kT16_list[c]} for c in range(NCORES)]
    r = bass_utils.run_bass_kernel_spmd(
        nc, in_maps, core_ids=list(range(NCORES)))
    LAST_EXEC_NS = getattr(r, "exec_time_ns", None)
    s = np.concatenate([r.results[c]["s"] for c in range(NCORES)], axis=1)
    # cm comes back [128, NCM, NQ] partition-major; -> [N, NCM] per core
    cm = np.concatenate(
        [r.results[c]["cm"].reshape(128, NCM, NQ).transpose(2, 0, 1)
         .reshape(N, NCM) for c in range(NCORES)], axis=1)
    return s, cm


def _device_scores_subprocess(xT16, kT16_list):
    """Run the device pass in a child process with a clean JAX env.

    Needed when the calling process pinned jax to CPU (run_bass_via_pjrt
    uses the default backend, which must be the neuron/axon one)."""
    import subprocess
    import tempfile

    with tempfile.TemporaryDirectory() as td:
        # fp8 dtypes don't round-trip through npz; ship uint8 views
        np.savez(os.path.join(td, "in.npz"), xT=xT16.view(np.uint8),
                 **{f"kT{c}": kT16_list[c].view(np.uint8)
                    for c in range(NCORES)})
        env = {k: v for k, v in os.environ.items()
               if k not in ("JAX_PLATFORMS", "BASS_TRACE")}
        script = (
            "import sys, numpy as np, ml_dtypes\n"
            f"sys.path.insert(0, {os.path.dirname(os.path.abspath(__file__))!r})\n"
            "import kernel as K\n"
            "f8 = ml_dtypes.float8_e4m3\n"
            f"d = np.load({os.path.join(td, 'in.npz')!r})\n"
            "s, cm = K._device_scores(d['xT'].view(f8), "
            f"[d[f'kT{{c}}'].view(f8) for c in range({NCORES})])\n"
            f"np.savez({os.path.join(td, 'out.npz')!r}, s=s, cm=cm)\n"
        )
        subprocess.run([sys.executable, "-c", script], check=True, env=env)
        out = np.load(os.path.join(td, "out.npz"))
        return out["s"], out["cm"]


# --- host orchestration ----------------------------------------------------
def _labels_from_candidates(x, keys_s, vals_s, k2_s, s16, cm16):
    # s16 holds pair-maxes: pair p covers keys (p//64)*128 + p%64 and +64
    nq = x.shape[0]
    HP = CMW // 2
    cmf = cm16.astype(np.float32)                      # [N, 512]
    k2min = k2_s[::CMW]                                # sorted asc -> min
    bound = cmf - 0.5 * k2min[None, :].astype(np.float32)
    ci = np.argpartition(-bound, TOP_CHUNKS, axis=1)[:, :TOP_CHUNKS]

    gp = (ci[:, :, None] * HP +
          np.arange(HP, dtype=np.int64)[None, None, :]).reshape(nq, -1)
    pv = np.take_along_axis(s16, gp, axis=1).astype(np.float32)
    # pair v upper bound: pairmax - 0.5*min(k2 of both keys) = k2 of lo key
    pair_lo_all = (np.arange(M // 2) // HP) * CMW + np.arange(M // 2) % HP
    vb = pv - 0.5 * k2_s[pair_lo_all[gp]].astype(np.float32)

    ri = np.argpartition(-vb, REFINE, axis=1)[:, :REFINE]
    rp = np.take_along_axis(gp, ri, axis=1)            # pairs [N, R]
    lo = pair_lo_all[rp]
    rglob = np.concatenate([lo, lo + HP], axis=1)      # keys [N, 2R]

    gk = keys_s[rglob]                                 # [N, 2R, 512] f32
    u_ex = np.einsum("qd,qrd->qr", x.astype(np.float32), gk,
                     optimize=True)
    v_ex = u_ex - 0.5 * k2_s[rglob].astype(np.float32)

    ti = np.argpartition(-v_ex, TOPK, axis=1)[:, :TOPK]
    top_glob = np.take_along_axis(rglob, ti, axis=1)
    cls = vals_s[top_glob]                             # [N, 10]

    counts = np.zeros((nq, NUM_CLASSES), np.int32)
    rows = np.arange(nq)
    for k in range(TOPK):
        np.add.at(counts, (rows, cls[:, k]), 1)
    return np.argmax(counts, axis=1)


def _labels_exact(x, keys, vals):
    u = x @ keys.T
    d2 = (keys * keys).sum(axis=1)[None, :].astype(np.float32) - 2.0 * u
    idx = np.argpartition(d2, TOPK, axis=1)[:, :TOPK]
    cls = vals[idx]
    counts = np.zeros((x.shape[0], NUM_CLASSES), np.int32)
    rows = np.arange(x.shape[0])
    for k in range(TOPK):
        np.add.at(counts, (rows, cls[:, k]), 1)
    return np.argmax(counts, axis=1)


def kernel(x, ver, keys, vals):
    x = np.asarray(x, dtype=np.float32)
    keys = np.asarray(keys, dtype=np.float32)
    vals = np.asarray(vals)

    k2 = np.einsum("md,md->m", keys, keys)             # [M] f32
    perm = np.argsort(k2, kind="stable")
    keys_s = keys[perm]
    vals_s = vals[perm]
    k2_s = k2[perm]

    import ml_dtypes
    f8 = ml_dtypes.float8_e4m3
    xT16 = np.ascontiguousarray(x.T).astype(f8)
    kT16 = [np.ascontiguousarray(keys_s[c * ML:(c + 1) * ML].T)
            .astype(f8) for c in range(NCORES)]

    try:
        import jax
        if jax.default_backend() == "cpu":
            # caller pinned jax to CPU; the bass PJRT path needs the
            # neuron backend as default, so run it in a clean child
            s16, cm16 = _device_scores_subprocess(xT16, kT16)
        else:
            s16, cm16 = _device_scores(xT16, kT16)
        # the device skips the last outer chunk's maxes; derive them from
        # the pair-max dump (local 128-key chunks 60..63 of each core)
        cm16 = np.ascontiguousarray(cm16)
        cm16.reshape(N, NCORES, NCM)[:, :, NCM - 4:] = (
            s16.reshape(N, NCORES, NCM, CMW // 2)[:, :, NCM - 4:, :]
            .max(axis=3))
        labels = _labels_from_candidates(x, keys_s, vals_s, k2_s, s16, cm16)
    except Exception:
        import traceback
        traceback.print_exc()
        print("kernel: device path failed, using host fallback",
              file=sys.stderr)
        labels = _labels_exact(x, keys, vals)
    return labels.astype(vals.dtype)


# revision 40
# speedup vs baseline: 1.0953x; 1.0953x over previous
"""KNN classifier (x[2048,512] vs keys[65536,512], top-10 mode vote).

Device (8 NeuronCores, datastore sharded 8192 keys/core):
  u = x @ keys_c.T in fp16 (PE, fp32 PSUM accum), dumped to HBM as fp16,
  plus per-128-key-chunk max of u (DVE) for cheap host-side candidate
  chunk selection.

Host: keys pre-sorted by |k|^2 so each 128-key chunk has a tight bias
range; chunks are selected by the provable bound max_u - 0.5*min|k|^2,
candidates narrowed with dumped fp16 scores, and the final top-10 is
re-ranked with exact f32 dot products so fp16 noise cannot flip labels.
"""

import os
import sys

import numpy as np

for _p in ("/opt/trn_rl_repo", "/root/.axon_site/_ro/trn_rl_repo"):
    if _p not in sys.path and os.path.isdir(_p):
        sys.path.append(_p)

N, D, M = 2048, 512, 65536
NCORES = 8
ML = M // NCORES          # 8192 keys per core
KT = D // 128             # 4 contraction tiles
NQ = N // 128             # 16 query tiles
CH = 512                  # psum chunk (one bank)
NCHUNK = ML // CH         # 16
CMW = 128                 # chunk-max granularity (keys per chunk)
NCM = ML // CMW           # 64 chunk maxes per core
TOPK = 10
NUM_CLASSES = 100
TOP_CHUNKS = 48           # candidate chunks per query (>= 12 guaranteed)
REFINE = 96               # exact-rescored candidates per query
QB = 4                    # query tiles per psum group (4 banks)

LAST_EXEC_NS = None


# --- walrus wait-cap workaround -------------------------------------------
# This toolchain's walrus caps semaphore waits per instruction at 1 (2 for
# EventSemaphore) but the Tile scheduler attaches more. Rewrite the BIR
# JSON before compile: excess waits move onto wait-only EventSemaphore
# instructions inserted before the oversubscribed instruction (same
# engine, in-order execution keeps semantics).
_WSPLIT = [0]


def _split_waits_json(bir):
    for fn in bir.get("functions", []):
        for bb in fn.get("blocks", []):
            out = []
            for inst in bb.get("instructions", []):
                si = inst.get("sync_info")
                if si:
                    waits = si.get("on_wait") or []
                    cap = 2 if inst.get("opcode") == "EventSemaphore" else 1
                    movable = [w for w in waits
                               if w.get("sync_type") == "semaphore"]
                    fixed = [w for w in waits
                             if w.get("sync_type") != "semaphore"]
                    room = max(cap - len(fixed), 0)
                    if len(movable) > room:
                        keep = movable[-room:] if room else []
                        excess = movable[:-room] if room else movable
                        si["on_wait"] = fixed + keep
                        for i in range(0, len(excess), 2):
                            _WSPLIT[0] += 1
                            out.append({
                                "engine": inst["engine"],
                                "ins": [], "outs": [],
                                "name": f"WSPLIT-{_WSPLIT[0]}",
                                "opcode": "EventSemaphore",
                                "debug": inst.get("debug", 0),
                                "sync_info": {"on_update": [],
                                              "on_wait": excess[i:i + 2]},
                            })
                out.append(inst)
            bb["instructions"] = out
    return bir


def _apply_tile_exit_patch():
    """Single-shot NEFF: skip the end-of-kernel semaphore clear + second
    barrier (the NEFF is loaded fresh per call, so sems never need
    resetting for a re-run). Keeps the allocator bookkeeping."""
    from concourse.tile import ScopedClock, TileContext

    if getattr(TileContext, "_exit_trimmed", False):
        return

    def _drain_and_barrier(self, tick_clock, wait_clock):
        drain_inst = self.nc.sync.drain()
        wait_clock.add_sem_waits(
            drain_inst.ins, ScopedClock({None: tick_clock.global_clock}))
        self.nc.all_engine_barrier()
        popped = self.nc._tile_sem_poison_stack.pop()
        assert popped is self._sem_poison
        sems = list(self.sems.allocated().values())
        if sems:
            nums = [s.num if hasattr(s, "num") else s for s in sems]
            self.nc._state.prepend_free_semaphores(nums)
            for poison_set in self.nc._tile_sem_poison_stack:
                poison_set.update(nums)

    TileContext._drain_and_barrier = _drain_and_barrier
    TileContext._exit_trimmed = True


def _apply_bir_patch():
    import json

    import concourse.bass2jax as b2j
    import concourse.bass_utils as bu

    if getattr(bu, "_wait_split_patched", False):
        return
    orig = bu.compile_bir_kernel

    def wrapper(bir_json, tmpdir, neff_name="file.neff", **kw):
        bir = json.loads(bytes(bir_json) if isinstance(
            bir_json, (bytes, bytearray)) else str(bir_json))
        bir = _split_waits_json(bir)
        return orig(json.dumps(bir).encode(), tmpdir, neff_name, **kw)

    bu.compile_bir_kernel = wrapper
    bu._wait_split_patched = True
    if getattr(b2j, "compile_bir_kernel", None) is orig:
        b2j.compile_bir_kernel = wrapper


# --- device kernel ---------------------------------------------------------
def _build_bass():
    import concourse.bass as bass
    import concourse.mybir as mybir
    from concourse.tile import TileContext

    nc = bass.Bass(trn_type="TRN2")
    f8 = mybir.dt.float8e4
    f16 = mybir.dt.float16
    f32 = mybir.dt.float32
    xT_d = nc.dram_tensor("xT", [D, N], f8, kind="ExternalInput")
    kT_d = nc.dram_tensor("kT", [D, ML], f8, kind="ExternalInput")
    # pair-maxes (keys b and b+64 of each 128-key chunk), halves the dump
    s_d = nc.dram_tensor("s", [N, ML // 2], f16, kind="ExternalOutput")
    # partition-major, chunk-major so each outer chunk flushes its own
    # 128x128B slice; host transposes back to [N, NCM]
    cm_d = nc.dram_tensor("cm", [128, NCM * NQ], f16, kind="ExternalOutput")

    CPC = CH // CMW  # chunk-maxes per psum chunk (4)

    NG = NQ // QB  # psum groups per chunk

    with TileContext(nc) as tc:
        with tc.tile_pool(name="inp", bufs=1) as inp, \
             tc.tile_pool(name="outp", bufs=2) as outp, \
             tc.tile_pool(name="ldr", bufs=2) as ldr, \
             tc.tile_pool(name="l1p", bufs=3) as l1p, \
             tc.tile_pool(name="ps", bufs=4, space="PSUM") as ps:
            # inputs arrive in fine slices, ordered so the first matmul's
            # operands (xt tile 0, key chunk 0) land first
            xts, kts = [], []

            def load_x(q):
                xq = inp.tile([128, KT, 128], f8, tag=f"xt{q}")
                nc.sync.dma_start(
                    out=xq[:],
                    in_=xT_d.ap()[:, q * 128:(q + 1) * 128]
                    .rearrange("(t p) n -> p t n", p=128))
                xts.append(xq)

            def load_k(c):
                kc = inp.tile([128, KT, CH], f8, tag=f"kt{c}")
                nc.sync.dma_start(
                    out=kc[:],
                    in_=kT_d.ap()[:, c * CH:(c + 1) * CH]
                    .rearrange("(t p) m -> p t m", p=128))
                kts.append(kc)

            load_x(0)
            load_k(0)
            load_k(1)
            for q in range(1, NQ):
                load_x(q)
            for c in range(2, NCHUNK):
                load_k(c)

            def ladder(dst, src, width, nq):
                # chunk-max via fp16 2x tensor_tensor max ladder:
                # width -> w/2 -> w/4 -> w/8, then a final reduce into dst.
                # Returns the first-stage (pair-max) tile, which doubles as
                # the score dump.
                cur = src
                w = width
                first = None
                while w > 16:
                    pool = l1p if w == width else ldr
                    nxt = pool.tile([128, nq, CPC, w // 2], f16,
                                    tag=f"l{w}_{nq}", name=f"lt{w}_{nq}")
                    nc.vector.tensor_tensor(
                        nxt[:], cur[:, :, :, 0:w // 2],
                        cur[:, :, :, w // 2:w], op=mybir.AluOpType.max)
                    if first is None:
                        first = nxt
                    cur, w = nxt, w // 2
                nc.vector.tensor_reduce(
                    dst, cur[:], axis=mybir.AxisListType.X,
                    op=mybir.AluOpType.max)
                return first

            HB = 2          # query tiles per psum half-tile (2 banks)
            NH = NQ // HB   # 8 half-tiles per chunk

            for c in range(NCHUNK):
                last = c == NCHUNK - 1
                # per-chunk score buffer + chunk-max tile (no cross-chunk
                # WAR on a shared accumulator)
                ev = outp.tile([128, NQ, CH], f16, tag="ev")
                cmc = ldr.tile([128, CPC, NQ], f16, tag="cmc")
                for h in range(NH):
                    acc = ps.tile([128, HB, CH], f32, tag="acc")
                    for j in range(HB):
                        q = h * HB + j
                        for t in range(KT // 2):
                            nc.tensor.matmul(
                                acc[:, j, :],
                                xts[q][:, 2 * t:2 * t + 2, :],
                                kts[c][:, 2 * t:2 * t + 2, :],
                                start=(t == 0), stop=(t == KT // 2 - 1),
                                perf_mode=mybir.MatmulPerfMode.DoubleRow)
                    # PSUM->SBUF fp16; DVE takes 1-2 of 8 (parity-based),
                    # Act the rest. Last chunk: all on Act so the DVE
                    # ladders drain the tail without queueing.
                    if not last and (h == 4 or (h == 5 and c % 2 == 0)):
                        nc.vector.tensor_copy(
                            ev[:, h * HB:(h + 1) * HB, :], acc[:])
                    else:
                        nc.scalar.copy(ev[:, h * HB:(h + 1) * HB, :],
                                       acc[:])
                    if last and h % 2 == 1:
                        g = h // 2
                        # tail: pair-max only, per 4 qtiles; the host
                        # derives this chunk's maxes from the dump
                        pm = l1p.tile([128, QB, CPC, CMW // 2], f16,
                                      tag="pml")
                        evg = ev[:, g * QB:(g + 1) * QB, :].rearrange(
                            "p q (a b) -> p q a b", b=CMW)
                        nc.vector.tensor_tensor(
                            pm[:], evg[:, :, :, 0:CMW // 2],
                            evg[:, :, :, CMW // 2:CMW],
                            op=mybir.AluOpType.max)
                        nc.sync.dma_start(
                            out=s_d.ap()[g * QB * 128:(g + 1) * QB * 128,
                                         c * (CH // 2):(c + 1) * (CH // 2)]
                            .rearrange("(q p) m -> p q m", p=128),
                            in_=pm[:].rearrange("p q a b -> p q (a b)"))
                if not last:
                    if c == NCHUNK - 2:
                        # shorten the tail: ladder per 4 qtiles so pieces
                        # start as their copies land
                        for g in range(NG):
                            pm = ladder(cmc[:, :, g * QB:(g + 1) * QB]
                                        .rearrange("p a q -> p q a"),
                                        ev[:, g * QB:(g + 1) * QB, :]
                                        .rearrange("p q (a b) -> p q a b",
                                                   b=CMW),
                                        CMW, QB)
                            nc.sync.dma_start(
                                out=s_d.ap()[
                                    g * QB * 128:(g + 1) * QB * 128,
                                    c * (CH // 2):(c + 1) * (CH // 2)]
                                .rearrange("(q p) m -> p q m", p=128),
                                in_=pm[:].rearrange("p q a b -> p q (a b)"))
                    else:
                        pm = ladder(cmc[:].rearrange("p a q -> p q a"),
                                    ev[:].rearrange("p q (a b) -> p q a b",
                                                    b=CMW),
                                    CMW, NQ)
                        nc.sync.dma_start(
                            out=s_d.ap()[:,
                                         c * (CH // 2):(c + 1) * (CH // 2)]
                            .rearrange("(q p) m -> p q m", p=128),
                            in_=pm[:].rearrange("p q a b -> p q (a b)"))
                    nc.sync.dma_start(
                        out=cm_d.ap()[:, c * CPC * NQ:(c + 1) * CPC * NQ],
                        in_=cmc[:].rearrange("p a q -> p (a q)"))
    return nc


def _device_scores(xT16, kT16_list):
    global LAST_EXEC_NS
    from concourse import bass_utils

    _apply_bir_patch()
    nc = _build_bass()
    in_maps = [{"xT": xT16, "kT": kT16_list[c]} for c in range(NCORES)]
    r = bass_utils.run_bass_kernel_spmd(
        nc, in_maps, core_ids=list(range(NCORES)))
    LAST_EXEC_NS = getattr(r, "exec_time_ns", None)
    s = np.concatenate([r.results[c]["s"] for c in range(NCORES)], axis=1)
    # cm comes back [128, NCM, NQ] partition-major; -> [N, NCM] per core
    cm = np.concatenate(
        [r.results[c]["cm"].reshape(128, NCM, NQ).transpose(2, 0, 1)
         .reshape(N, NCM) for c in range(NCORES)], axis=1)
    return s, cm


def _device_scores_subprocess(xT16, kT16_list):
    """Run the device pass in a child process with a clean JAX env.

    Needed when the calling process pinned jax to CPU (run_bass_via_pjrt
    uses the default backend, which must be the neuron/axon one)."""
    import subprocess
    import tempfile

    with tempfile.TemporaryDirectory() as td:
        # fp8 dtypes don't round-trip through npz; ship uint8 views
        np.savez(os.path.join(td, "in.npz"), xT=xT16.view(np.uint8),
                 **{f"kT{c}": kT16_list[c].view(np.uint8)
                    for c in range(NCORES)})
        env = {k: v for k, v in os.environ.items()
               if k not in ("JAX_PLATFORMS", "BASS_TRACE")}
        script = (
            "import sys, numpy as np, ml_dtypes\n"
            f"sys.path.insert(0, {os.path.dirname(os.path.abspath(__file__))!r})\n"
            "import kernel as K\n"
            "f8 = ml_dtypes.float8_e4m3\n"
            f"d = np.load({os.path.join(td, 'in.npz')!r})\n"
            "s, cm = K._device_scores(d['xT'].view(f8), "
            f"[d[f'kT{{c}}'].view(f8) for c in range({NCORES})])\n"
            f"np.savez({os.path.join(td, 'out.npz')!r}, s=s, cm=cm)\n"
        )
        subprocess.run([sys.executable, "-c", script], check=True, env=env)
        out = np.load(os.path.join(td, "out.npz"))
        return out["s"], out["cm"]


# --- host orchestration ----------------------------------------------------
def _labels_from_candidates(x, keys_s, vals_s, k2_s, s16, cm16):
    # s16 holds pair-maxes: pair p covers keys (p//64)*128 + p%64 and +64
    nq = x.shape[0]
    HP = CMW // 2
    cmf = cm16.astype(np.float32)                      # [N, 512]
    k2min = k2_s[::CMW]                                # sorted asc -> min
    bound = cmf - 0.5 * k2min[None, :].astype(np.float32)
    ci = np.argpartition(-bound, TOP_CHUNKS, axis=1)[:, :TOP_CHUNKS]

    gp = (ci[:, :, None] * HP +
          np.arange(HP, dtype=np.int64)[None, None, :]).reshape(nq, -1)
    pv = np.take_along_axis(s16, gp, axis=1).astype(np.float32)
    # pair v upper bound: pairmax - 0.5*min(k2 of both keys) = k2 of lo key
    pair_lo_all = (np.arange(M // 2) // HP) * CMW + np.arange(M // 2) % HP
    vb = pv - 0.5 * k2_s[pair_lo_all[gp]].astype(np.float32)

    ri = np.argpartition(-vb, REFINE, axis=1)[:, :REFINE]
    rp = np.take_along_axis(gp, ri, axis=1)            # pairs [N, R]
    lo = pair_lo_all[rp]
    rglob = np.concatenate([lo, lo + HP], axis=1)      # keys [N, 2R]

    gk = keys_s[rglob]                                 # [N, 2R, 512] f32
    u_ex = np.einsum("qd,qrd->qr", x.astype(np.float32), gk,
                     optimize=True)
    v_ex = u_ex - 0.5 * k2_s[rglob].astype(np.float32)

    ti = np.argpartition(-v_ex, TOPK, axis=1)[:, :TOPK]
    top_glob = np.take_along_axis(rglob, ti, axis=1)
    cls = vals_s[top_glob]                             # [N, 10]

    counts = np.zeros((nq, NUM_CLASSES), np.int32)
    rows = np.arange(nq)
    for k in range(TOPK):
        np.add.at(counts, (rows, cls[:, k]), 1)
    return np.argmax(counts, axis=1)


def _labels_exact(x, keys, vals):
    u = x @ keys.T
    d2 = (keys * keys).sum(axis=1)[None, :].astype(np.float32) - 2.0 * u
    idx = np.argpartition(d2, TOPK, axis=1)[:, :TOPK]
    cls = vals[idx]
    counts = np.zeros((x.shape[0], NUM_CLASSES), np.int32)
    rows = np.arange(x.shape[0])
    for k in range(TOPK):
        np.add.at(counts, (rows, cls[:, k]), 1)
    return np.argmax(counts, axis=1)


def kernel(x, ver, keys, vals):
    x = np.asarray(x, dtype=np.float32)
    keys = np.asarray(keys, dtype=np.float32)
    vals = np.asarray(vals)

    k2 = np.einsum("md,md->m", keys, keys)             # [M] f32
    perm = np.argsort(k2, kind="stable")
    keys_s = keys[perm]
    vals_s = vals[perm]
    k2_s = k2[perm]

    import ml_dtypes
    f8 = ml_dtypes.float8_e4m3
    xT16 = np.ascontiguousarray(x.T).astype(f8)
    kT16 = [np.ascontiguousarray(keys_s[c * ML:(c + 1) * ML].T)
            .astype(f8) for c in range(NCORES)]

    try:
        import jax
        if jax.default_backend() == "cpu":
            # caller pinned jax to CPU; the bass PJRT path needs the
            # neuron backend as default, so run it in a clean child
            s16, cm16 = _device_scores_subprocess(xT16, kT16)
        else:
            s16, cm16 = _device_scores(xT16, kT16)
        # the device skips the last outer chunk's maxes; derive them from
        # the pair-max dump (local 128-key chunks 60..63 of each core)
        cm16 = np.ascontiguousarray(cm16)
        cm16.reshape(N, NCORES, NCM)[:, :, NCM - 4:] = (
            s16.reshape(N, NCORES, NCM, CMW // 2)[:, :, NCM - 4:, :]
            .max(axis=3))
        labels = _labels_from_candidates(x, keys_s, vals_s, k2_s, s16, cm16)
    except Exception:
        import traceback
        traceback.print_exc()
        print("kernel: device path failed, using host fallback",
              file=sys.stderr)
        labels = _labels_exact(x, keys, vals)
    return labels.astype(vals.dtype)
